# revision 22
# baseline (speedup 1.0000x reference)
"""Trainium2 Bass kernel for nn_CrossAttention (B=4, C=256, H=W=64).

Per (batch, branch) the computation is an independent cross-attention:
    f = Wf @ other + bf          [32, 4096]
    g = Wg @ own   + bg          [32, 4096]
    h = Wh @ own   + bh          [256, 4096]
    S = f^T @ g                  [4096, 4096]
    att = softmax(S, axis=-1)
    sa[c, m] = sum_n h[c, n] * att[n, m]
    out = gamma * sa + own

B*2 = 8 independent problems -> one per NeuronCore (pure SPMD).

Factorization: att[n,m] = E[n,m]/Z[n] with E = exp(S - K0), Z = rowsum(E)
(accum_out of the exp activation), so sa = (h/Z)^T @ E with E computed once
in bf16.  The fixed K0 cancels in E/Z and guards fp32 overflow.

Schedule: 64 slots, one exp chunk [128n x 2048m] per slot (8 n-tile groups
x 8 chunks).  ACT streams exps back-to-back (the ~144us critical path);
the PE's trailing work each slot is sa accumulation for completed groups.
sa accumulates in PSUM across multi-group windows ({0,1},{2,3},{4}..{7})
before a single DVE eviction per (window, m-block, half) that also folds
gamma (STT: sa_sb += gamma*psum).  The residual add uses the fp16 input
(no fp32 copy of x is ever loaded).  E tiles and the oth input share one
rotating SBUF pool sized so exp never stalls on buffer reuse.
"""

import os
import sys

for _p in ("/opt/trn_rl_repo", "/opt/pypackages"):
    if _p not in sys.path:
        sys.path.insert(0, _p)

os.environ.setdefault("JAX_PLATFORMS", "")

import numpy as np

import concourse.bacc as bacc
import concourse.tile as tile
from concourse import mybir

F32 = mybir.dt.float32
F16 = mybir.dt.float16
BF16 = mybir.dt.bfloat16
AF = mybir.ActivationFunctionType
ALU = mybir.AluOpType

B, C, H, W = 4, 256, 64, 64
N = H * W            # 4096 pixels
C8 = C // 8          # 32
NT = N // 128        # 32 n-tiles
NGROUP = 4           # n-tiles per group (Z granularity)
NG = NT // NGROUP    # 8 groups
MB = 512             # m-block (one PSUM bank of fp32)
NMB = N // MB        # 8 m-blocks
K0 = 40.0            # constant subtracted inside exp (cancels in softmax)
IN_T = 2048          # input tile columns
# exp chunks per n-tile: m cols [0:1536), [1536:3072), [3072:4096).
# 1536 = 3 PSUM banks -> two S buffers (6 banks) + 2 conv/sa banks = 8,
# so the S matmuls double-buffer and the ACT stream never waits on them.
CHUNK_COLS = [(0, 1536), (1536, 1536), (3072, 1024)]
NCH = len(CHUNK_COLS)           # 3 chunks per n-tile
SLOTS = NT * NCH                # 96 chunk slots
# sa accumulation windows (groups whose contribution sums in PSUM before
# one eviction); later windows are single groups so their sa work lands
# inside the exp stream instead of after it.
WINDOWS = [[0, 1], [2, 3], [4], [5], [6], [7]]


def build_bass():
    nc = bacc.Bacc()

    own_d = nc.dram_tensor("own16", [C, N], F16, kind="ExternalInput")
    oth_d = nc.dram_tensor("oth16", [C, N], F16, kind="ExternalInput")
    # wf/wg are pre-tiled 4x along their free dim so the conv matmul writes
    # all four partition-quad replicas directly (no SBUF-SBUF copy DMAs).
    wf_d = nc.dram_tensor("wf_t", [C, 128], F16, kind="ExternalInput")
    wg_d = nc.dram_tensor("wg_t", [C, 128], F16, kind="ExternalInput")
    wh_d = nc.dram_tensor("wh_t", [C, C], F16, kind="ExternalInput")
    bf_d = nc.dram_tensor("bf_rep", [128, 1], F32, kind="ExternalInput")
    bg_d = nc.dram_tensor("bg_rep", [128, 1], F32, kind="ExternalInput")
    bh_d = nc.dram_tensor("bh_row", [1, C], F16, kind="ExternalInput")
    gm_d = nc.dram_tensor("gamma_rep", [128, 1], F32, kind="ExternalInput")
    on_d = nc.dram_tensor("ones_row", [1, 128], F16, kind="ExternalInput")
    k0_d = nc.dram_tensor("k0_col", [128, 1], F32, kind="ExternalInput")
    out_d = nc.dram_tensor("out", [C, N], F32, kind="ExternalOutput")

    with tile.TileContext(nc) as tc:
        with (
            tc.tile_pool(name="singles", bufs=1) as singles,
            tc.tile_pool(name="own", bufs=1) as ownp,
            tc.tile_pool(name="othp", bufs=1) as othp,
            tc.tile_pool(name="e3", bufs=28) as e3pool,
            tc.tile_pool(name="e2", bufs=14) as e2pool,
            tc.tile_pool(name="fp", bufs=8) as fpool,
            tc.tile_pool(name="zpool", bufs=4) as zpool,
            tc.tile_pool(name="outp", bufs=2) as outp,
            # ps_c (2 banks) serves the convs early and the sa units later;
            # ps_s double-buffers the 3-bank S chunks (6 banks).
            tc.tile_pool(name="ps_c", bufs=2, space="PSUM") as ps_c,
            tc.tile_pool(name="ps_s", bufs=2, space="PSUM") as ps_s,
        ):
            # ---- DMA priority: what the first convs need comes first ----
            wf_sb = [singles.tile([128, 128], F16, name=f"wf{k}") for k in range(2)]
            wg_sb = [singles.tile([128, 128], F16, name=f"wg{k}") for k in range(2)]
            wh_sb = [singles.tile([128, C], F16, name=f"wh{k}") for k in range(2)]
            bf_sb = singles.tile([128, 1], F32)
            bg_sb = singles.tile([128, 1], F32)
            bh_sb = singles.tile([1, C], F16)
            gm_sb = singles.tile([128, 1], F32)
            ones_sb = singles.tile([1, 128], F16)
            k0_sb = singles.tile([128, 1], F32)
            own_sb = [[ownp.tile([128, IN_T], F16, name=f"own{k}_{t}")
                       for t in range(2)] for k in range(2)]
            oth_sb = [[othp.tile([128, IN_T], F16, name=f"oth{k}_{t}")
                       for t in range(2)] for k in range(2)]

            for k in range(2):
                nc.sync.dma_start(out=wg_sb[k], in_=wg_d[128 * k:128 * (k + 1), :])
                nc.sync.dma_start(out=wf_sb[k], in_=wf_d[128 * k:128 * (k + 1), :])
            nc.sync.dma_start(out=bg_sb, in_=bg_d[:, :])
            nc.sync.dma_start(out=bf_sb, in_=bf_d[:, :])
            for k in range(2):
                nc.sync.dma_start(out=own_sb[k][0], in_=own_d[128 * k:128 * (k + 1), 0:IN_T])
            for k in range(2):
                nc.sync.dma_start(out=oth_sb[k][0], in_=oth_d[128 * k:128 * (k + 1), 0:IN_T])
            for k in range(2):
                nc.sync.dma_start(out=own_sb[k][1], in_=own_d[128 * k:128 * (k + 1), IN_T:N])
            nc.sync.dma_start(out=k0_sb, in_=k0_d[:, :])
            for k in range(2):
                nc.sync.dma_start(out=wh_sb[k], in_=wh_d[128 * k:128 * (k + 1), :])
            nc.sync.dma_start(out=bh_sb, in_=bh_d[:, :])
            nc.sync.dma_start(out=ones_sb, in_=on_d[:, :])
            nc.sync.dma_start(out=gm_sb, in_=gm_d[:, :])
            for k in range(2):
                nc.sync.dma_start(out=oth_sb[k][1], in_=oth_d[128 * k:128 * (k + 1), IN_T:N])

            # g blocks static (live for the whole kernel); f blocks rotate
            # (f_q[g] is only read during group g's stats chunks).
            g_q = [singles.tile([128, MB], F16, name=f"g{nb}") for nb in range(NMB)]
            f_q = {}
            sa_sb = [singles.tile([128, N], F16, name=f"sa{k}") for k in range(2)]
            hxz = [singles.tile([128, C], BF16, name=f"hxz{i}") for i in range(NT)]
            e_t = {}   # (g, a, h) -> E tile [128, HALF] bf16

            def conv_g(nb):
                ps = ps_c.tile([128, MB], F32, tag="c")
                for k in range(2):
                    nc.tensor.matmul(
                        out=ps,
                        lhsT=wg_sb[k],
                        rhs=own_sb[k][nb // 4][:, MB * (nb % 4):MB * (nb % 4 + 1)],
                        start=(k == 0),
                        stop=(k == 1),
                    )
                nc.vector.tensor_scalar(
                    out=g_q[nb], in0=ps,
                    scalar1=bg_sb[:, 0:1], scalar2=None, op0=ALU.add)

            def conv_f(nb):
                dst = fpool.tile([128, MB], F16, name=f"f{nb}", tag="f")
                f_q[nb] = dst
                ps = ps_c.tile([128, MB], F32, tag="c")
                for k in range(2):
                    nc.tensor.matmul(
                        out=ps,
                        lhsT=wf_sb[k],
                        rhs=oth_sb[k][nb // 4][:, MB * (nb % 4):MB * (nb % 4 + 1)],
                        start=(k == 0),
                        stop=(k == 1),
                    )
                nc.vector.tensor_scalar(
                    out=dst, in0=ps,
                    scalar1=bf_sb[:, 0:1], scalar2=None, op0=ALU.add)

            def conv_h(i):
                t, o = (128 * i) // IN_T, (128 * i) % IN_T
                ph = ps_c.tile([128, C], F32, tag="c")
                nc.tensor.matmul(out=ph, lhsT=ones_sb, rhs=bh_sb,
                                 start=True, stop=False)
                for k in range(2):
                    nc.tensor.matmul(
                        out=ph,
                        lhsT=own_sb[k][t][:, o:o + 128],
                        rhs=wh_sb[k],
                        start=False,
                        stop=(k == 1),
                    )
                nc.vector.tensor_copy(out=hxz[i], in_=ph)

            def stats_chunk(i, c, zp):
                """S chunk (n-tile i, m chunk c) -> exp -> E; Z part via
                DVE (c=0,2) or gpsimd (c=1) in-place identity accum."""
                g, a = i // NGROUP, i % NGROUP
                o = 128 * a
                m0, cols = CHUNK_COLS[c]
                sp = ps_s.tile([128, cols], F32, tag="s")
                for mb in range(m0 // MB, (m0 + cols) // MB):
                    j = mb % 4
                    nc.tensor.matmul(
                        out=sp[:, MB * mb - m0:MB * (mb + 1) - m0],
                        lhsT=f_q[g][32 * j:32 * (j + 1), o:o + 128],
                        rhs=g_q[mb][32 * j:32 * (j + 1), :],
                        start=True,
                        stop=True,
                        tile_position=(32 * j, 0),
                    )
                pool = e3pool if cols == 1536 else e2pool
                et = pool.tile([128, cols], BF16, name=f"e{i}_{c}", tag="e")
                e_t[(i, c)] = et
                nc.scalar.activation(out=et, in_=sp, func=AF.Exp,
                                     bias=k0_sb[:, 0:1])
                zcol = zp[:, NCH * a + c:NCH * a + c + 1]
                nc.vector.tensor_scalar(out=et, in0=et, scalar1=1.0, scalar2=0.0,
                                        op0=ALU.mult, op1=ALU.add, accum_out=zcol)

            def zprep(g, zp):
                """Z = sum of the three chunk sums; hxz *= 1/Z (in place)."""
                zt = zpool.tile([128, NGROUP], F32, tag="zt")
                rz = zpool.tile([128, NGROUP], F32, tag="rz")
                nc.vector.tensor_add(out=zt, in0=zp[:, 0:12:3], in1=zp[:, 1:12:3])
                nc.vector.tensor_add(out=zt, in0=zt, in1=zp[:, 2:12:3])
                nc.vector.reciprocal(out=rz, in_=zt)
                for a in range(NGROUP):
                    nc.vector.tensor_scalar(
                        out=hxz[NGROUP * g + a],
                        in0=hxz[NGROUP * g + a],
                        scalar1=rz[:, a:a + 1],
                        scalar2=None,
                        op0=ALU.mult,
                    )

            def sa_unit(w, mb, ch):
                """One window's contribution to sa[:, mb block], half ch."""
                c = mb * MB // 1536          # which E chunk holds this m-block
                m0 = mb * MB - CHUNK_COLS[c][0]
                groups = WINDOWS[w]
                pa = ps_c.tile([128, MB], F32, tag="c")
                nmm = 4 * len(groups)
                k = 0
                for g in groups:
                    for a in range(NGROUP):
                        nc.tensor.matmul(
                            out=pa,
                            lhsT=hxz[NGROUP * g + a][:, 128 * ch:128 * (ch + 1)],
                            rhs=e_t[(NGROUP * g + a, c)][:, m0:m0 + MB],
                            start=(k == 0),
                            stop=(k == nmm - 1),
                        )
                        k += 1
                dst = sa_sb[ch][:, MB * mb:MB * (mb + 1)]
                if w == 0:
                    nc.vector.tensor_scalar(
                        out=dst, in0=pa,
                        scalar1=gm_sb[:, 0:1], scalar2=None, op0=ALU.mult)
                else:
                    nc.vector.scalar_tensor_tensor(
                        out=dst, in0=pa, scalar=gm_sb[:, 0:1], in1=dst,
                        op0=ALU.mult, op1=ALU.add)
                if w == len(WINDOWS) - 1:
                    ot = outp.tile([128, MB], F32, tag="ot")
                    nc.vector.tensor_add(
                        out=ot, in0=dst,
                        in1=own_sb[ch][mb // 4][:, MB * (mb % 4):MB * (mb % 4 + 1)])
                    nc.sync.dma_start(
                        out=out_d[128 * ch:128 * (ch + 1), MB * mb:MB * (mb + 1)],
                        in_=ot)

            # ---- slot schedule ----
            # upfront: chunk (0,0) needs g0-2 + f0; the rest of the first
            # n-tile's chunks need all of g.
            for nb in range(3):
                conv_g(nb)
            conv_f(0)
            for nb in range(3, NMB):
                conv_g(nb)

            # early filler: h and f convs, 2 per slot.  h convs for group
            # g's n-tiles must land before zprep(g) (slot 12g+12); f_q[g]
            # before slot 12g.
            filler = [("h", i) for i in range(4)] \
                   + [("f", nb) for nb in range(1, 4)] \
                   + [("h", i) for i in range(4, 16)] \
                   + [("f", 4), ("f", 5)] \
                   + [("h", i) for i in range(16, 24)] \
                   + [("f", 6), ("f", 7)] \
                   + [("h", i) for i in range(24, NT)]
            FILL_PER_SLOT = 2

            # sa units become available per window after its last zprep.
            ready_slot = {w: 12 * (max(gs) + 1) for w, gs in enumerate(WINDOWS)}
            unit_queue = []
            for w in range(len(WINDOWS)):
                for mb in range(NMB):
                    for ch in range(2):
                        unit_queue.append((ready_slot[w], w, mb, ch))
            unit_queue.sort(key=lambda u: u[0])
            uq_pos = 0

            def emit_trailing(s, budget):
                """Emit trailing PE work for slot s."""
                nonlocal uq_pos
                for _ in range(FILL_PER_SLOT):
                    if filler:
                        kind, arg = filler.pop(0)
                        if kind == "g":
                            conv_g(arg)
                        elif kind == "f":
                            conv_f(arg)
                        else:
                            conv_h(arg)
                done = 0
                while done < budget and uq_pos < len(unit_queue):
                    rs, w, mb, ch = unit_queue[uq_pos]
                    if rs > s:
                        break
                    sa_unit(w, mb, ch)
                    uq_pos += 1
                    done += 1

            zps = {}
            for i in range(NT):
                g, a = i // NGROUP, i % NGROUP
                if a == 0:
                    zps[g] = zpool.tile([128, NCH * NGROUP], F32, tag="zp",
                                        name=f"zp{g}")
                    if g > 0:
                        zprep(g - 1, zps.pop(g - 1))
                for c in range(NCH):
                    s = NCH * i + c
                    emit_trailing(s, 1 if s < 60 else 2)
                    stats_chunk(i, c, zps[g])
            zprep(NG - 1, zps.pop(NG - 1))
            # tail: remaining units (last window)
            while uq_pos < len(unit_queue):
                _, w, mb, ch = unit_queue[uq_pos]
                sa_unit(w, mb, ch)
                uq_pos += 1

    if not nc.is_finalized():
        nc.finalize()
    return nc


_NC_CACHE = None


def _get_nc():
    global _NC_CACHE
    if _NC_CACHE is None:
        _NC_CACHE = build_bass()
    return _NC_CACHE


def make_in_maps(**inputs):
    """Build the 8 per-core input maps (core 2b = x-branch, 2b+1 = y-branch)."""
    f = lambda a: np.ascontiguousarray(np.asarray(a), dtype=np.float32)
    h16 = lambda a: np.ascontiguousarray(np.asarray(a), dtype=np.float16)
    x16 = h16(inputs["x"]).reshape(B, C, N)
    y16 = h16(inputs["y"]).reshape(B, C, N)
    Wfx, bfx = h16(inputs["Wfx"]), f(inputs["bfx"])
    Wgx, bgx = h16(inputs["Wgx"]), f(inputs["bgx"])
    Whx, bhx = h16(inputs["Whx"]), h16(inputs["bhx"])
    Wfy, bfy = h16(inputs["Wfy"]), f(inputs["bfy"])
    Wgy, bgy = h16(inputs["Wgy"]), f(inputs["bgy"])
    Why, bhy = h16(inputs["Why"]), h16(inputs["bhy"])
    gamma = f(inputs["gamma"])

    rep4 = lambda b: np.ascontiguousarray(np.tile(b, 4).reshape(128, 1))
    gam = np.ascontiguousarray(np.broadcast_to(gamma.reshape(1, 1), (128, 1)))

    c16 = lambda a: np.ascontiguousarray(a, dtype=np.float16)
    rep4c = lambda w: c16(np.tile(w.T, (1, 4)))   # [C, C8] -> [C, 128]
    branch = {
        "x": dict(
            wf_t=rep4c(Wfy), wg_t=rep4c(Wgx), wh_t=c16(Whx.T),
            bf_rep=rep4(bfy), bg_rep=rep4(bgx), bh_row=c16(bhx.reshape(1, C)),
        ),
        "y": dict(
            wf_t=rep4c(Wfx), wg_t=rep4c(Wgy), wh_t=c16(Why.T),
            bf_rep=rep4(bfx), bg_rep=rep4(bgy), bh_row=c16(bhy.reshape(1, C)),
        ),
    }

    ones_row = np.ones((1, 128), np.float16)
    k0_col = np.full((128, 1), -K0, np.float32)
    in_maps = []
    for b in range(B):
        in_maps.append(dict(own16=x16[b], oth16=y16[b],
                            gamma_rep=gam, ones_row=ones_row, k0_col=k0_col,
                            **branch["x"]))
        in_maps.append(dict(own16=y16[b], oth16=x16[b],
                            gamma_rep=gam, ones_row=ones_row, k0_col=k0_col,
                            **branch["y"]))
    return in_maps


def kernel(**inputs):
    from concourse.bass_utils import run_bass_kernel_spmd

    nc = _get_nc()
    in_maps = make_in_maps(**inputs)
    res = run_bass_kernel_spmd(nc, in_maps, list(range(8))).results
    out_x = np.stack([res[2 * b]["out"] for b in range(B)]).reshape(B, C, H, W)
    out_y = np.stack([res[2 * b + 1]["out"] for b in range(B)]).reshape(B, C, H, W)
    return (out_x, out_y)


# revision 26
# speedup vs baseline: 1.1155x; 1.1155x over previous
"""Trainium2 Bass kernel for nn_CrossAttention (B=4, C=256, H=W=64).

Per (batch, branch) the computation is an independent cross-attention:
    f = Wf @ other + bf          [32, 4096]
    g = Wg @ own   + bg          [32, 4096]
    h = Wh @ own   + bh          [256, 4096]
    S = f^T @ g                  [4096, 4096]
    att = softmax(S, axis=-1)
    sa[c, m] = sum_n h[c, n] * att[n, m]
    out = gamma * sa + own

B*2 = 8 independent problems -> one per NeuronCore (pure SPMD).

Factorization: att[n,m] = E[n,m]/Z[n] with E = exp(S - K0), Z = rowsum(E)
(accum_out of the exp activation), so sa = (h/Z)^T @ E with E computed once
in bf16.  The fixed K0 cancels in E/Z and guards fp32 overflow.

Schedule: 64 slots, one exp chunk [128n x 2048m] per slot (8 n-tile groups
x 8 chunks).  ACT streams exps back-to-back (the ~144us critical path);
the PE's trailing work each slot is sa accumulation for completed groups.
sa accumulates in PSUM across multi-group windows ({0,1},{2,3},{4}..{7})
before a single DVE eviction per (window, m-block, half) that also folds
gamma (STT: sa_sb += gamma*psum).  The residual add uses the fp16 input
(no fp32 copy of x is ever loaded).  E tiles and the oth input share one
rotating SBUF pool sized so exp never stalls on buffer reuse.
"""

import os
import sys

for _p in ("/opt/trn_rl_repo", "/opt/pypackages"):
    if _p not in sys.path:
        sys.path.insert(0, _p)

os.environ.setdefault("JAX_PLATFORMS", "")

import numpy as np

import concourse.bacc as bacc
import concourse.tile as tile
from concourse import mybir

F32 = mybir.dt.float32
F16 = mybir.dt.float16
BF16 = mybir.dt.bfloat16
AF = mybir.ActivationFunctionType
ALU = mybir.AluOpType

B, C, H, W = 4, 256, 64, 64
N = H * W            # 4096 pixels
C8 = C // 8          # 32
NT = N // 128        # 32 n-tiles
NGROUP = 4           # n-tiles per group (Z granularity)
NG = NT // NGROUP    # 8 groups
MB = 512             # m-block (one PSUM bank of fp32)
NMB = N // MB        # 8 m-blocks
K0 = 40.0            # constant subtracted inside exp (cancels in softmax)
IN_T = 2048          # input tile columns
# exp chunks per n-tile: m cols [0:1536), [1536:3072), [3072:4096).
# 1536 = 3 PSUM banks -> two S buffers (6 banks) + 2 conv/sa banks = 8,
# so the S matmuls double-buffer and the ACT stream never waits on them.
CHUNK_COLS = [(0, 1536), (1536, 1536), (3072, 1024)]
NCH = len(CHUNK_COLS)           # 3 chunks per n-tile
SLOTS = NT * NCH                # 96 chunk slots
# sa accumulation windows (groups whose contribution sums in PSUM before
# one eviction); later windows are single groups so their sa work lands
# inside the exp stream instead of after it.
WINDOWS = [[0, 1], [2, 3], [4], [5], [6], [7]]


def build_bass():
    nc = bacc.Bacc()

    own_d = nc.dram_tensor("own16", [C, N], F16, kind="ExternalInput")
    oth_d = nc.dram_tensor("oth16", [C, N], F16, kind="ExternalInput")
    # wf/wg are pre-tiled 4x along their free dim so the conv matmul writes
    # all four partition-quad replicas directly (no SBUF-SBUF copy DMAs).
    wf_d = nc.dram_tensor("wf_t", [C, 128], F16, kind="ExternalInput")
    wg_d = nc.dram_tensor("wg_t", [C, 128], F16, kind="ExternalInput")
    wh_d = nc.dram_tensor("wh_t", [C, C], F16, kind="ExternalInput")
    bf_d = nc.dram_tensor("bf_rep", [128, 1], F32, kind="ExternalInput")
    bg_d = nc.dram_tensor("bg_rep", [128, 1], F32, kind="ExternalInput")
    bh_d = nc.dram_tensor("bh_row", [1, C], F16, kind="ExternalInput")
    gm_d = nc.dram_tensor("gamma_rep", [128, 1], F32, kind="ExternalInput")
    on_d = nc.dram_tensor("ones_row", [1, 128], F16, kind="ExternalInput")
    k0_d = nc.dram_tensor("k0_col", [128, 1], F32, kind="ExternalInput")
    out_d = nc.dram_tensor("out", [C, N], F32, kind="ExternalOutput")

    with tile.TileContext(nc) as tc:
        with (
            tc.tile_pool(name="singles", bufs=1) as singles,
            tc.tile_pool(name="own", bufs=1) as ownp,
            tc.tile_pool(name="othp", bufs=1) as othp,
            tc.tile_pool(name="e3", bufs=28) as e3pool,
            tc.tile_pool(name="e2", bufs=14) as e2pool,
            tc.tile_pool(name="fp", bufs=8) as fpool,
            tc.tile_pool(name="zpool", bufs=4) as zpool,
            tc.tile_pool(name="outp", bufs=2) as outp,
            # ps_c (2 banks) serves the convs early and the sa units later;
            # ps_s double-buffers the 3-bank S chunks (6 banks).
            tc.tile_pool(name="ps_c", bufs=2, space="PSUM") as ps_c,
            tc.tile_pool(name="ps_s", bufs=2, space="PSUM") as ps_s,
        ):
            # ---- DMA priority: what the first convs need comes first ----
            wf_sb = [singles.tile([128, 128], F16, name=f"wf{k}") for k in range(2)]
            wg_sb = [singles.tile([128, 128], F16, name=f"wg{k}") for k in range(2)]
            wh_sb = [singles.tile([128, C], F16, name=f"wh{k}") for k in range(2)]
            bf_sb = singles.tile([128, 1], F32)
            bg_sb = singles.tile([128, 1], F32)
            bh_sb = singles.tile([1, C], F16)
            gm_sb = singles.tile([128, 1], F32)
            ones_sb = singles.tile([1, 128], F16)
            k0_sb = singles.tile([128, 1], F32)
            # inputs split [0:512 | 512:2048 | 2048:4096] so the first conv
            # only waits on a 128KB transfer; own/oth/weights issue on three
            # separate DMA queues (sync/vector/scalar) to parallelize the
            # ~620ns-per-DMA issue serialization.
            own_sb = [(ownp.tile([128, 512], F16, name=f"ownA{k}"),
                       ownp.tile([128, 1536], F16, name=f"ownB{k}"),
                       ownp.tile([128, IN_T], F16, name=f"ownC{k}"))
                      for k in range(2)]
            oth_sb = [(othp.tile([128, 512], F16, name=f"othA{k}"),
                       othp.tile([128, 1536], F16, name=f"othB{k}"),
                       othp.tile([128, IN_T], F16, name=f"othC{k}"))
                      for k in range(2)]

            def islice(tr, c0, w):
                if c0 < 512:
                    return tr[0][:, c0:c0 + w]
                if c0 < IN_T:
                    return tr[1][:, c0 - 512:c0 - 512 + w]
                return tr[2][:, c0 - IN_T:c0 - IN_T + w]

            for k in range(2):
                nc.sync.dma_start(out=wg_sb[k], in_=wg_d[128 * k:128 * (k + 1), :])
                nc.scalar.dma_start(out=wf_sb[k], in_=wf_d[128 * k:128 * (k + 1), :])
                nc.scalar.dma_start(out=wh_sb[k], in_=wh_d[128 * k:128 * (k + 1), :])
            nc.sync.dma_start(out=bg_sb, in_=bg_d[:, :])
            nc.scalar.dma_start(out=bf_sb, in_=bf_d[:, :])
            nc.scalar.dma_start(out=bh_sb, in_=bh_d[:, :])
            for k in range(2):
                nc.sync.dma_start(out=own_sb[k][0], in_=own_d[128 * k:128 * (k + 1), 0:512])
                nc.scalar.dma_start(out=oth_sb[k][0], in_=oth_d[128 * k:128 * (k + 1), 0:512])
            for k in range(2):
                nc.sync.dma_start(out=own_sb[k][1], in_=own_d[128 * k:128 * (k + 1), 512:IN_T])
                nc.scalar.dma_start(out=oth_sb[k][1], in_=oth_d[128 * k:128 * (k + 1), 512:IN_T])
            nc.scalar.dma_start(out=ones_sb, in_=on_d[:, :])
            nc.scalar.dma_start(out=k0_sb, in_=k0_d[:, :])
            nc.scalar.dma_start(out=gm_sb, in_=gm_d[:, :])
            for k in range(2):
                nc.sync.dma_start(out=own_sb[k][2], in_=own_d[128 * k:128 * (k + 1), IN_T:N])
                nc.scalar.dma_start(out=oth_sb[k][2], in_=oth_d[128 * k:128 * (k + 1), IN_T:N])

            # g blocks static (live for the whole kernel); f blocks rotate
            # (f_q[g] is only read during group g's stats chunks).
            g_q = [singles.tile([128, MB], F16, name=f"g{nb}") for nb in range(NMB)]
            f_q = {}
            sa_sb = [singles.tile([128, N], F16, name=f"sa{k}") for k in range(2)]
            hxz = [singles.tile([128, C], BF16, name=f"hxz{i}") for i in range(NT)]
            e_t = {}   # (g, a, h) -> E tile [128, HALF] bf16

            def conv_g(nb):
                ps = ps_c.tile([128, MB], F32, tag="c")
                for k in range(2):
                    nc.tensor.matmul(
                        out=ps,
                        lhsT=wg_sb[k],
                        rhs=islice(own_sb[k], MB * nb, MB),
                        start=(k == 0),
                        stop=(k == 1),
                    )
                nc.vector.tensor_scalar(
                    out=g_q[nb], in0=ps,
                    scalar1=bg_sb[:, 0:1], scalar2=None, op0=ALU.add)

            def conv_f(nb):
                dst = fpool.tile([128, MB], F16, name=f"f{nb}", tag="f")
                f_q[nb] = dst
                ps = ps_c.tile([128, MB], F32, tag="c")
                for k in range(2):
                    nc.tensor.matmul(
                        out=ps,
                        lhsT=wf_sb[k],
                        rhs=islice(oth_sb[k], MB * nb, MB),
                        start=(k == 0),
                        stop=(k == 1),
                    )
                nc.vector.tensor_scalar(
                    out=dst, in0=ps,
                    scalar1=bf_sb[:, 0:1], scalar2=None, op0=ALU.add)

            def conv_h(i):
                ph = ps_c.tile([128, C], F32, tag="c")
                nc.tensor.matmul(out=ph, lhsT=ones_sb, rhs=bh_sb,
                                 start=True, stop=False)
                for k in range(2):
                    nc.tensor.matmul(
                        out=ph,
                        lhsT=islice(own_sb[k], 128 * i, 128),
                        rhs=wh_sb[k],
                        start=False,
                        stop=(k == 1),
                    )
                nc.vector.tensor_copy(out=hxz[i], in_=ph)

            def stats_chunk(i, c, zp):
                """S chunk (n-tile i, m chunk c) -> exp -> E; Z part via
                DVE (c=0,2) or gpsimd (c=1) in-place identity accum."""
                g, a = i // NGROUP, i % NGROUP
                o = 128 * a
                m0, cols = CHUNK_COLS[c]
                sp = ps_s.tile([128, cols], F32, tag="s")
                for mb in range(m0 // MB, (m0 + cols) // MB):
                    j = mb % 4
                    nc.tensor.matmul(
                        out=sp[:, MB * mb - m0:MB * (mb + 1) - m0],
                        lhsT=f_q[g][32 * j:32 * (j + 1), o:o + 128],
                        rhs=g_q[mb][32 * j:32 * (j + 1), :],
                        start=True,
                        stop=True,
                        tile_position=(32 * j, 0),
                    )
                pool = e3pool if cols == 1536 else e2pool
                et = pool.tile([128, cols], BF16, name=f"e{i}_{c}", tag="e")
                e_t[(i, c)] = et
                nc.scalar.activation(out=et, in_=sp, func=AF.Exp,
                                     bias=k0_sb[:, 0:1],
                                     accum_out=zp[:, NCH * a + c:NCH * a + c + 1])

            def zprep(g, zp):
                """Z = sum of the three chunk sums; hxz *= 1/Z (in place)."""
                zt = zpool.tile([128, NGROUP], F32, tag="zt")
                rz = zpool.tile([128, NGROUP], F32, tag="rz")
                nc.vector.tensor_add(out=zt, in0=zp[:, 0:12:3], in1=zp[:, 1:12:3])
                nc.vector.tensor_add(out=zt, in0=zt, in1=zp[:, 2:12:3])
                nc.vector.reciprocal(out=rz, in_=zt)
                for a in range(NGROUP):
                    nc.vector.tensor_scalar(
                        out=hxz[NGROUP * g + a],
                        in0=hxz[NGROUP * g + a],
                        scalar1=rz[:, a:a + 1],
                        scalar2=None,
                        op0=ALU.mult,
                    )

            def sa_unit(w, mb, ch):
                """One window's contribution to sa[:, mb block], half ch."""
                c = mb * MB // 1536          # which E chunk holds this m-block
                m0 = mb * MB - CHUNK_COLS[c][0]
                groups = WINDOWS[w]
                pa = ps_c.tile([128, MB], F32, tag="c")
                nmm = 4 * len(groups)
                k = 0
                for g in groups:
                    for a in range(NGROUP):
                        nc.tensor.matmul(
                            out=pa,
                            lhsT=hxz[NGROUP * g + a][:, 128 * ch:128 * (ch + 1)],
                            rhs=e_t[(NGROUP * g + a, c)][:, m0:m0 + MB],
                            start=(k == 0),
                            stop=(k == nmm - 1),
                        )
                        k += 1
                dst = sa_sb[ch][:, MB * mb:MB * (mb + 1)]
                if w == 0:
                    nc.vector.tensor_scalar(
                        out=dst, in0=pa,
                        scalar1=gm_sb[:, 0:1], scalar2=None, op0=ALU.mult)
                else:
                    nc.vector.scalar_tensor_tensor(
                        out=dst, in0=pa, scalar=gm_sb[:, 0:1], in1=dst,
                        op0=ALU.mult, op1=ALU.add)
                if w == len(WINDOWS) - 1:
                    ot = outp.tile([128, MB], F32, tag="ot")
                    nc.vector.tensor_add(
                        out=ot, in0=dst,
                        in1=islice(own_sb[ch], MB * mb, MB))
                    nc.sync.dma_start(
                        out=out_d[128 * ch:128 * (ch + 1), MB * mb:MB * (mb + 1)],
                        in_=ot)

            # ---- slot schedule ----
            # upfront: chunk (0,0) needs g0-2 + f0; the rest of the first
            # n-tile's chunks need all of g.
            for nb in range(3):
                conv_g(nb)
            conv_f(0)
            for nb in range(3, NMB):
                conv_g(nb)

            # early filler: h and f convs, 2 per slot.  h convs for group
            # g's n-tiles must land before zprep(g) (slot 12g+12); f_q[g]
            # before slot 12g.
            filler = [("h", i) for i in range(4)] \
                   + [("f", nb) for nb in range(1, 4)] \
                   + [("h", i) for i in range(4, 16)] \
                   + [("f", 4), ("f", 5)] \
                   + [("h", i) for i in range(16, 24)] \
                   + [("f", 6), ("f", 7)] \
                   + [("h", i) for i in range(24, NT)]
            FILL_PER_SLOT = 2

            # sa units become available per window after its last zprep.
            ready_slot = {w: 12 * (max(gs) + 1) for w, gs in enumerate(WINDOWS)}
            unit_queue = []
            for w in range(len(WINDOWS)):
                for mb in range(NMB):
                    for ch in range(2):
                        unit_queue.append((ready_slot[w], w, mb, ch))
            unit_queue.sort(key=lambda u: u[0])
            uq_pos = 0

            def emit_trailing(s, budget):
                """Emit trailing PE work for slot s."""
                nonlocal uq_pos
                for _ in range(FILL_PER_SLOT):
                    if filler:
                        kind, arg = filler.pop(0)
                        if kind == "g":
                            conv_g(arg)
                        elif kind == "f":
                            conv_f(arg)
                        else:
                            conv_h(arg)
                done = 0
                while done < budget and uq_pos < len(unit_queue):
                    rs, w, mb, ch = unit_queue[uq_pos]
                    if rs > s:
                        break
                    sa_unit(w, mb, ch)
                    uq_pos += 1
                    done += 1

            zps = {}
            for i in range(NT):
                g, a = i // NGROUP, i % NGROUP
                if a == 0:
                    zps[g] = zpool.tile([128, NCH * NGROUP], F32, tag="zp",
                                        name=f"zp{g}")
                    if g > 0:
                        zprep(g - 1, zps.pop(g - 1))
                for c in range(NCH):
                    s = NCH * i + c
                    emit_trailing(s, 1 if s < 60 else 2)
                    stats_chunk(i, c, zps[g])
            zprep(NG - 1, zps.pop(NG - 1))
            # tail: remaining units (last window)
            while uq_pos < len(unit_queue):
                _, w, mb, ch = unit_queue[uq_pos]
                sa_unit(w, mb, ch)
                uq_pos += 1

    if not nc.is_finalized():
        nc.finalize()
    return nc


_NC_CACHE = None


def _get_nc():
    global _NC_CACHE
    if _NC_CACHE is None:
        _NC_CACHE = build_bass()
    return _NC_CACHE


def make_in_maps(**inputs):
    """Build the 8 per-core input maps (core 2b = x-branch, 2b+1 = y-branch)."""
    f = lambda a: np.ascontiguousarray(np.asarray(a), dtype=np.float32)
    h16 = lambda a: np.ascontiguousarray(np.asarray(a), dtype=np.float16)
    x16 = h16(inputs["x"]).reshape(B, C, N)
    y16 = h16(inputs["y"]).reshape(B, C, N)
    Wfx, bfx = h16(inputs["Wfx"]), f(inputs["bfx"])
    Wgx, bgx = h16(inputs["Wgx"]), f(inputs["bgx"])
    Whx, bhx = h16(inputs["Whx"]), h16(inputs["bhx"])
    Wfy, bfy = h16(inputs["Wfy"]), f(inputs["bfy"])
    Wgy, bgy = h16(inputs["Wgy"]), f(inputs["bgy"])
    Why, bhy = h16(inputs["Why"]), h16(inputs["bhy"])
    gamma = f(inputs["gamma"])

    rep4 = lambda b: np.ascontiguousarray(np.tile(b, 4).reshape(128, 1))
    gam = np.ascontiguousarray(np.broadcast_to(gamma.reshape(1, 1), (128, 1)))

    c16 = lambda a: np.ascontiguousarray(a, dtype=np.float16)
    rep4c = lambda w: c16(np.tile(w.T, (1, 4)))   # [C, C8] -> [C, 128]
    branch = {
        "x": dict(
            wf_t=rep4c(Wfy), wg_t=rep4c(Wgx), wh_t=c16(Whx.T),
            bf_rep=rep4(bfy), bg_rep=rep4(bgx), bh_row=c16(bhx.reshape(1, C)),
        ),
        "y": dict(
            wf_t=rep4c(Wfx), wg_t=rep4c(Wgy), wh_t=c16(Why.T),
            bf_rep=rep4(bfx), bg_rep=rep4(bgy), bh_row=c16(bhy.reshape(1, C)),
        ),
    }

    ones_row = np.ones((1, 128), np.float16)
    k0_col = np.full((128, 1), -K0, np.float32)
    in_maps = []
    for b in range(B):
        in_maps.append(dict(own16=x16[b], oth16=y16[b],
                            gamma_rep=gam, ones_row=ones_row, k0_col=k0_col,
                            **branch["x"]))
        in_maps.append(dict(own16=y16[b], oth16=x16[b],
                            gamma_rep=gam, ones_row=ones_row, k0_col=k0_col,
                            **branch["y"]))
    return in_maps


def kernel(**inputs):
    from concourse.bass_utils import run_bass_kernel_spmd

    nc = _get_nc()
    in_maps = make_in_maps(**inputs)
    res = run_bass_kernel_spmd(nc, in_maps, list(range(8))).results
    out_x = np.stack([res[2 * b]["out"] for b in range(B)]).reshape(B, C, H, W)
    out_y = np.stack([res[2 * b + 1]["out"] for b in range(B)]).reshape(B, C, H, W)
    return (out_x, out_y)


# revision 29
# speedup vs baseline: 1.2671x; 1.1359x over previous
"""Trainium2 Bass kernel for nn_CrossAttention (B=4, C=256, H=W=64).

Per (batch, branch) the computation is an independent cross-attention:
    f = Wf @ other + bf          [32, 4096]
    g = Wg @ own   + bg          [32, 4096]
    h = Wh @ own   + bh          [256, 4096]
    S = f^T @ g                  [4096, 4096]
    att = softmax(S, axis=-1)
    sa[c, m] = sum_n h[c, n] * att[n, m]
    out = gamma * sa + own

B*2 = 8 independent problems -> one per NeuronCore (pure SPMD).

Factorization: att[n,m] = E[n,m]/Z[n] with E = exp(S - K0), Z = rowsum(E)
(accum_out of the exp activation), so sa = (h/Z)^T @ E with E computed once
in bf16.  The fixed K0 cancels in E/Z and guards fp32 overflow.

Schedule: 64 slots, one exp chunk [128n x 2048m] per slot (8 n-tile groups
x 8 chunks).  ACT streams exps back-to-back (the ~144us critical path);
the PE's trailing work each slot is sa accumulation for completed groups.
sa accumulates in PSUM across multi-group windows ({0,1},{2,3},{4}..{7})
before a single DVE eviction per (window, m-block, half) that also folds
gamma (STT: sa_sb += gamma*psum).  The residual add uses the fp16 input
(no fp32 copy of x is ever loaded).  E tiles and the oth input share one
rotating SBUF pool sized so exp never stalls on buffer reuse.
"""

import os
import sys

for _p in ("/opt/trn_rl_repo", "/opt/pypackages"):
    if _p not in sys.path:
        sys.path.insert(0, _p)

os.environ.setdefault("JAX_PLATFORMS", "")

import numpy as np

import concourse.bacc as bacc
import concourse.tile as tile
from concourse import mybir

F32 = mybir.dt.float32
F16 = mybir.dt.float16
BF16 = mybir.dt.bfloat16
AF = mybir.ActivationFunctionType
ALU = mybir.AluOpType

B, C, H, W = 4, 256, 64, 64
N = H * W            # 4096 pixels
C8 = C // 8          # 32
NT = N // 128        # 32 n-tiles
NGROUP = 4           # n-tiles per group (Z granularity)
NG = NT // NGROUP    # 8 groups
MB = 512             # m-block (one PSUM bank of fp32)
NMB = N // MB        # 8 m-blocks
K0 = 40.0            # constant subtracted inside exp (cancels in softmax)
IN_T = 2048          # input tile columns
# exp chunks per n-tile: m cols [0:1536), [1536:3072), [3072:4096).
# 1536 = 3 PSUM banks -> two S buffers (6 banks) + 2 conv/sa banks = 8,
# so the S matmuls double-buffer and the ACT stream never waits on them.
CHUNK_COLS = [(0, 1536), (1536, 1536), (3072, 1024)]
NCH = len(CHUNK_COLS)           # 3 chunks per n-tile
SLOTS = NT * NCH                # 96 chunk slots
# sa accumulation windows (groups whose contribution sums in PSUM before
# one eviction); later windows are single groups so their sa work lands
# inside the exp stream instead of after it.
WINDOWS = [[0, 1], [2, 3], [4, 5], [6], [7]]


def build_bass():
    nc = bacc.Bacc()

    own_d = nc.dram_tensor("own16", [C, N], F16, kind="ExternalInput")
    oth_d = nc.dram_tensor("oth16", [C, N], F16, kind="ExternalInput")
    # wf/wg are pre-tiled 4x along their free dim so the conv matmul writes
    # all four partition-quad replicas directly (no SBUF-SBUF copy DMAs).
    wf_d = nc.dram_tensor("wf_t", [C, 128], F16, kind="ExternalInput")
    wg_d = nc.dram_tensor("wg_t", [C, 128], F16, kind="ExternalInput")
    wh_d = nc.dram_tensor("wh_t", [C, C], F16, kind="ExternalInput")
    bf_d = nc.dram_tensor("bf_rep", [128, 1], F32, kind="ExternalInput")
    bg_d = nc.dram_tensor("bg_rep", [128, 1], F32, kind="ExternalInput")
    bh_d = nc.dram_tensor("bh_row", [1, C], F16, kind="ExternalInput")
    gm_d = nc.dram_tensor("gamma_rep", [128, 1], F32, kind="ExternalInput")
    on_d = nc.dram_tensor("ones_row", [1, 128], F16, kind="ExternalInput")
    k0_d = nc.dram_tensor("k0_col", [128, 1], F32, kind="ExternalInput")
    out_d = nc.dram_tensor("out", [C, N], F32, kind="ExternalOutput")

    with tile.TileContext(nc) as tc:
        with (
            tc.tile_pool(name="singles", bufs=1) as singles,
            tc.tile_pool(name="own", bufs=1) as ownp,
            tc.tile_pool(name="othp", bufs=1) as othp,
            tc.tile_pool(name="e3", bufs=28) as e3pool,
            tc.tile_pool(name="e2", bufs=14) as e2pool,
            tc.tile_pool(name="fp", bufs=8) as fpool,
            tc.tile_pool(name="zpool", bufs=4) as zpool,
            tc.tile_pool(name="outp", bufs=2) as outp,
            # ps_c (2 banks) serves the convs early and the sa units later;
            # ps_s double-buffers the 3-bank S chunks (6 banks).
            tc.tile_pool(name="ps_c", bufs=2, space="PSUM") as ps_c,
            tc.tile_pool(name="ps_s", bufs=2, space="PSUM") as ps_s,
        ):
            # ---- DMA priority: what the first convs need comes first ----
            wf_sb = [singles.tile([128, 128], F16, name=f"wf{k}") for k in range(2)]
            wg_sb = [singles.tile([128, 128], F16, name=f"wg{k}") for k in range(2)]
            wh_sb = [singles.tile([128, C], F16, name=f"wh{k}") for k in range(2)]
            bf_sb = singles.tile([128, 1], F32)
            bg_sb = singles.tile([128, 1], F32)
            bh_sb = singles.tile([1, C], F16)
            gm_sb = singles.tile([128, 1], F32)
            ones_sb = singles.tile([1, 128], F16)
            k0_sb = singles.tile([128, 1], F32)
            # inputs split [0:1536 | 1536:4096] (first piece covers the g/f
            # convs the first exp chunk needs); the k=0 half issues on the
            # sync queue and k=1 on the scalar queue so the ~620ns-per-DMA
            # issue serialization halves.
            own_sb = [(ownp.tile([128, 1536], F16, name=f"ownA{k}"),
                       ownp.tile([128, N - 1536], F16, name=f"ownB{k}"))
                      for k in range(2)]
            oth_sb = [(othp.tile([128, 1536], F16, name=f"othA{k}"),
                       othp.tile([128, N - 1536], F16, name=f"othB{k}"))
                      for k in range(2)]

            def islice(tr, c0, w):
                if c0 < 1536:
                    return tr[0][:, c0:c0 + w]
                return tr[1][:, c0 - 1536:c0 - 1536 + w]

            dmaq = [nc.sync, nc.scalar]
            for k in range(2):
                dmaq[k].dma_start(out=wg_sb[k], in_=wg_d[128 * k:128 * (k + 1), :])
            for k in range(2):
                dmaq[k].dma_start(out=own_sb[k][0], in_=own_d[128 * k:128 * (k + 1), 0:1536])
            nc.sync.dma_start(out=bg_sb, in_=bg_d[:, :])
            nc.scalar.dma_start(out=bf_sb, in_=bf_d[:, :])
            for k in range(2):
                dmaq[k].dma_start(out=wf_sb[k], in_=wf_d[128 * k:128 * (k + 1), :])
            for k in range(2):
                dmaq[k].dma_start(out=oth_sb[k][0], in_=oth_d[128 * k:128 * (k + 1), 0:1536])
            nc.sync.dma_start(out=k0_sb, in_=k0_d[:, :])
            for k in range(2):
                dmaq[k].dma_start(out=own_sb[k][1], in_=own_d[128 * k:128 * (k + 1), 1536:N])
            for k in range(2):
                dmaq[k].dma_start(out=wh_sb[k], in_=wh_d[128 * k:128 * (k + 1), :])
            nc.sync.dma_start(out=bh_sb, in_=bh_d[:, :])
            nc.scalar.dma_start(out=ones_sb, in_=on_d[:, :])
            nc.sync.dma_start(out=gm_sb, in_=gm_d[:, :])
            for k in range(2):
                dmaq[k].dma_start(out=oth_sb[k][1], in_=oth_d[128 * k:128 * (k + 1), 1536:N])

            # g blocks static (live for the whole kernel); f blocks rotate
            # (f_q[g] is only read during group g's stats chunks).
            g_q = [singles.tile([128, MB], F16, name=f"g{nb}") for nb in range(NMB)]
            f_q = {}
            sa_sb = [singles.tile([128, N], F16, name=f"sa{k}") for k in range(2)]
            hxz = [singles.tile([128, C], BF16, name=f"hxz{i}") for i in range(NT)]
            e_t = {}   # (g, a, h) -> E tile [128, HALF] bf16

            def conv_g(nb):
                ps = ps_c.tile([128, MB], F32, tag="c")
                for k in range(2):
                    nc.tensor.matmul(
                        out=ps,
                        lhsT=wg_sb[k],
                        rhs=islice(own_sb[k], MB * nb, MB),
                        start=(k == 0),
                        stop=(k == 1),
                    )
                nc.vector.tensor_scalar(
                    out=g_q[nb], in0=ps,
                    scalar1=bg_sb[:, 0:1], scalar2=None, op0=ALU.add)

            def conv_f(nb):
                dst = fpool.tile([128, MB], F16, name=f"f{nb}", tag="f")
                f_q[nb] = dst
                ps = ps_c.tile([128, MB], F32, tag="c")
                for k in range(2):
                    nc.tensor.matmul(
                        out=ps,
                        lhsT=wf_sb[k],
                        rhs=islice(oth_sb[k], MB * nb, MB),
                        start=(k == 0),
                        stop=(k == 1),
                    )
                nc.vector.tensor_scalar(
                    out=dst, in0=ps,
                    scalar1=bf_sb[:, 0:1], scalar2=None, op0=ALU.add)

            def conv_h(i):
                ph = ps_c.tile([128, C], F32, tag="c")
                nc.tensor.matmul(out=ph, lhsT=ones_sb, rhs=bh_sb,
                                 start=True, stop=False)
                for k in range(2):
                    nc.tensor.matmul(
                        out=ph,
                        lhsT=islice(own_sb[k], 128 * i, 128),
                        rhs=wh_sb[k],
                        start=False,
                        stop=(k == 1),
                    )
                nc.vector.tensor_copy(out=hxz[i], in_=ph)

            def stats_chunk(i, c, zp):
                """S chunk (n-tile i, m chunk c) -> exp -> E; Z part via
                DVE (c=0,2) or gpsimd (c=1) in-place identity accum."""
                g, a = i // NGROUP, i % NGROUP
                o = 128 * a
                m0, cols = CHUNK_COLS[c]
                sp = ps_s.tile([128, cols], F32, tag="s")
                for mb in range(m0 // MB, (m0 + cols) // MB):
                    j = mb % 4
                    nc.tensor.matmul(
                        out=sp[:, MB * mb - m0:MB * (mb + 1) - m0],
                        lhsT=f_q[g][32 * j:32 * (j + 1), o:o + 128],
                        rhs=g_q[mb][32 * j:32 * (j + 1), :],
                        start=True,
                        stop=True,
                        tile_position=(32 * j, 0),
                    )
                pool = e3pool if cols == 1536 else e2pool
                et = pool.tile([128, cols], BF16, name=f"e{i}_{c}", tag="e")
                e_t[(i, c)] = et
                nc.scalar.activation(out=et, in_=sp, func=AF.Exp,
                                     bias=k0_sb[:, 0:1],
                                     accum_out=zp[:, NCH * a + c:NCH * a + c + 1])

            def zprep(g, zp):
                """Z = sum of the three chunk sums; hxz *= 1/Z (in place)."""
                zt = zpool.tile([128, NGROUP], F32, tag="zt")
                rz = zpool.tile([128, NGROUP], F32, tag="rz")
                nc.vector.tensor_add(out=zt, in0=zp[:, 0:12:3], in1=zp[:, 1:12:3])
                nc.vector.tensor_add(out=zt, in0=zt, in1=zp[:, 2:12:3])
                nc.vector.reciprocal(out=rz, in_=zt)
                for a in range(NGROUP):
                    nc.vector.tensor_scalar(
                        out=hxz[NGROUP * g + a],
                        in0=hxz[NGROUP * g + a],
                        scalar1=rz[:, a:a + 1],
                        scalar2=None,
                        op0=ALU.mult,
                    )

            def sa_unit(w, mb, ch):
                """One window's contribution to sa[:, mb block], half ch."""
                c = mb * MB // 1536          # which E chunk holds this m-block
                m0 = mb * MB - CHUNK_COLS[c][0]
                groups = WINDOWS[w]
                pa = ps_c.tile([128, MB], F32, tag="c")
                nmm = 4 * len(groups)
                k = 0
                for g in groups:
                    for a in range(NGROUP):
                        nc.tensor.matmul(
                            out=pa,
                            lhsT=hxz[NGROUP * g + a][:, 128 * ch:128 * (ch + 1)],
                            rhs=e_t[(NGROUP * g + a, c)][:, m0:m0 + MB],
                            start=(k == 0),
                            stop=(k == nmm - 1),
                        )
                        k += 1
                dst = sa_sb[ch][:, MB * mb:MB * (mb + 1)]
                if w == 0:
                    nc.vector.tensor_scalar(
                        out=dst, in0=pa,
                        scalar1=gm_sb[:, 0:1], scalar2=None, op0=ALU.mult)
                else:
                    nc.vector.scalar_tensor_tensor(
                        out=dst, in0=pa, scalar=gm_sb[:, 0:1], in1=dst,
                        op0=ALU.mult, op1=ALU.add)
                if w == len(WINDOWS) - 1:
                    ot = outp.tile([128, MB], F32, tag="ot")
                    nc.vector.tensor_add(
                        out=ot, in0=dst,
                        in1=islice(own_sb[ch], MB * mb, MB))
                    nc.sync.dma_start(
                        out=out_d[128 * ch:128 * (ch + 1), MB * mb:MB * (mb + 1)],
                        in_=ot)

            # ---- slot schedule ----
            # upfront: chunk (0,0) needs g0-2 + f0; the rest of the first
            # n-tile's chunks need all of g.
            for nb in range(3):
                conv_g(nb)
            conv_f(0)
            for nb in range(3, NMB):
                conv_g(nb)

            # early filler: h and f convs, 2 per slot.  h convs for group
            # g's n-tiles must land before zprep(g) (slot 12g+12); f_q[g]
            # before slot 12g.
            filler = [("h", i) for i in range(4)] \
                   + [("f", nb) for nb in range(1, 4)] \
                   + [("h", i) for i in range(4, 16)] \
                   + [("f", 4), ("f", 5)] \
                   + [("h", i) for i in range(16, 24)] \
                   + [("f", 6), ("f", 7)] \
                   + [("h", i) for i in range(24, NT)]
            FILL_PER_SLOT = 2

            # sa units become available per window after its last zprep.
            # Pace them with a cumulative quota that spreads each window's
            # 16 units across the slots until the NEXT window opens (plus a
            # few slots of overlap) so the PE never drains dry right at a
            # window boundary (which re-throttles the HAM clock).
            ready_slot = {w: 12 * (max(gs) + 1) for w, gs in enumerate(WINDOWS)}
            unit_queue = []
            for w in range(len(WINDOWS)):
                for mb in range(NMB):
                    for ch in range(2):
                        unit_queue.append((ready_slot[w], w, mb, ch))
            unit_queue.sort(key=lambda u: u[0])
            uq_pos = 0
            rs_list = sorted(ready_slot.values()) + [SLOTS]
            spans = {}
            for w, r in ready_slot.items():
                nxt = min([x for x in rs_list if x > r] + [SLOTS])
                spans[w] = max(1, nxt + 4 - r)

            def quota(s):
                q = 0.0
                for w, r in ready_slot.items():
                    if s >= r:
                        q += 16.0 * min(1.0, (s - r) / spans[w])
                return q

            def emit_trailing(s):
                """Emit trailing PE work for slot s."""
                nonlocal uq_pos
                for _ in range(FILL_PER_SLOT):
                    if filler:
                        kind, arg = filler.pop(0)
                        if kind == "g":
                            conv_g(arg)
                        elif kind == "f":
                            conv_f(arg)
                        else:
                            conv_h(arg)
                while uq_pos < min(quota(s), len(unit_queue)):
                    rs, w, mb, ch = unit_queue[uq_pos]
                    if rs > s:
                        break
                    sa_unit(w, mb, ch)
                    uq_pos += 1

            zps = {}
            for i in range(NT):
                g, a = i // NGROUP, i % NGROUP
                if a == 0:
                    zps[g] = zpool.tile([128, NCH * NGROUP], F32, tag="zp",
                                        name=f"zp{g}")
                    if g > 0:
                        zprep(g - 1, zps.pop(g - 1))
                for c in range(NCH):
                    s = NCH * i + c
                    emit_trailing(s)
                    stats_chunk(i, c, zps[g])
            zprep(NG - 1, zps.pop(NG - 1))
            # tail: remaining units (last window)
            while uq_pos < len(unit_queue):
                _, w, mb, ch = unit_queue[uq_pos]
                sa_unit(w, mb, ch)
                uq_pos += 1

    if not nc.is_finalized():
        nc.finalize()
    return nc


_NC_CACHE = None


def _get_nc():
    global _NC_CACHE
    if _NC_CACHE is None:
        _NC_CACHE = build_bass()
    return _NC_CACHE


def make_in_maps(**inputs):
    """Build the 8 per-core input maps (core 2b = x-branch, 2b+1 = y-branch)."""
    f = lambda a: np.ascontiguousarray(np.asarray(a), dtype=np.float32)
    h16 = lambda a: np.ascontiguousarray(np.asarray(a), dtype=np.float16)
    x16 = h16(inputs["x"]).reshape(B, C, N)
    y16 = h16(inputs["y"]).reshape(B, C, N)
    Wfx, bfx = h16(inputs["Wfx"]), f(inputs["bfx"])
    Wgx, bgx = h16(inputs["Wgx"]), f(inputs["bgx"])
    Whx, bhx = h16(inputs["Whx"]), h16(inputs["bhx"])
    Wfy, bfy = h16(inputs["Wfy"]), f(inputs["bfy"])
    Wgy, bgy = h16(inputs["Wgy"]), f(inputs["bgy"])
    Why, bhy = h16(inputs["Why"]), h16(inputs["bhy"])
    gamma = f(inputs["gamma"])

    rep4 = lambda b: np.ascontiguousarray(np.tile(b, 4).reshape(128, 1))
    gam = np.ascontiguousarray(np.broadcast_to(gamma.reshape(1, 1), (128, 1)))

    c16 = lambda a: np.ascontiguousarray(a, dtype=np.float16)
    rep4c = lambda w: c16(np.tile(w.T, (1, 4)))   # [C, C8] -> [C, 128]
    branch = {
        "x": dict(
            wf_t=rep4c(Wfy), wg_t=rep4c(Wgx), wh_t=c16(Whx.T),
            bf_rep=rep4(bfy), bg_rep=rep4(bgx), bh_row=c16(bhx.reshape(1, C)),
        ),
        "y": dict(
            wf_t=rep4c(Wfx), wg_t=rep4c(Wgy), wh_t=c16(Why.T),
            bf_rep=rep4(bfx), bg_rep=rep4(bgy), bh_row=c16(bhy.reshape(1, C)),
        ),
    }

    ones_row = np.ones((1, 128), np.float16)
    k0_col = np.full((128, 1), -K0, np.float32)
    in_maps = []
    for b in range(B):
        in_maps.append(dict(own16=x16[b], oth16=y16[b],
                            gamma_rep=gam, ones_row=ones_row, k0_col=k0_col,
                            **branch["x"]))
        in_maps.append(dict(own16=y16[b], oth16=x16[b],
                            gamma_rep=gam, ones_row=ones_row, k0_col=k0_col,
                            **branch["y"]))
    return in_maps


def kernel(**inputs):
    from concourse.bass_utils import run_bass_kernel_spmd

    nc = _get_nc()
    in_maps = make_in_maps(**inputs)
    res = run_bass_kernel_spmd(nc, in_maps, list(range(8))).results
    out_x = np.stack([res[2 * b]["out"] for b in range(B)]).reshape(B, C, H, W)
    out_y = np.stack([res[2 * b + 1]["out"] for b in range(B)]).reshape(B, C, H, W)
    return (out_x, out_y)


# revision 33
# speedup vs baseline: 1.3049x; 1.0299x over previous
"""Trainium2 Bass kernel for nn_CrossAttention (B=4, C=256, H=W=64).

Per (batch, branch) the computation is an independent cross-attention:
    f = Wf @ other + bf          [32, 4096]
    g = Wg @ own   + bg          [32, 4096]
    h = Wh @ own   + bh          [256, 4096]
    S = f^T @ g                  [4096, 4096]
    att = softmax(S, axis=-1)
    sa[c, m] = sum_n h[c, n] * att[n, m]
    out = gamma * sa + own

B*2 = 8 independent problems -> one per NeuronCore (pure SPMD).

Factorization: att[n,m] = E[n,m]/Z[n] with E = exp(S - K0), Z = rowsum(E)
(accum_out of the exp activation), so sa = (h/Z)^T @ E with E computed once
in bf16.  The fixed K0 cancels in E/Z and guards fp32 overflow.

Schedule: 64 slots, one exp chunk [128n x 2048m] per slot (8 n-tile groups
x 8 chunks).  ACT streams exps back-to-back (the ~144us critical path);
the PE's trailing work each slot is sa accumulation for completed groups.
sa accumulates in PSUM across multi-group windows ({0,1},{2,3},{4}..{7})
before a single DVE eviction per (window, m-block, half) that also folds
gamma (STT: sa_sb += gamma*psum).  The residual add uses the fp16 input
(no fp32 copy of x is ever loaded).  E tiles and the oth input share one
rotating SBUF pool sized so exp never stalls on buffer reuse.
"""

import os
import sys

for _p in ("/opt/trn_rl_repo", "/opt/pypackages"):
    if _p not in sys.path:
        sys.path.insert(0, _p)

os.environ.setdefault("JAX_PLATFORMS", "")

import numpy as np

import concourse.bacc as bacc
import concourse.tile as tile
from concourse import mybir

F32 = mybir.dt.float32
F16 = mybir.dt.float16
BF16 = mybir.dt.bfloat16
AF = mybir.ActivationFunctionType
ALU = mybir.AluOpType

B, C, H, W = 4, 256, 64, 64
N = H * W            # 4096 pixels
C8 = C // 8          # 32
NT = N // 128        # 32 n-tiles
NGROUP = 4           # n-tiles per group (Z granularity)
NG = NT // NGROUP    # 8 groups
MB = 512             # m-block (one PSUM bank of fp32)
NMB = N // MB        # 8 m-blocks
K0 = 40.0            # constant subtracted inside exp (cancels in softmax)
IN_T = 2048          # input tile columns
# exp chunks per n-tile: m cols [0:1536), [1536:3072), [3072:4096).
# 1536 = 3 PSUM banks -> two S buffers (6 banks) + 2 conv/sa banks = 8,
# so the S matmuls double-buffer and the ACT stream never waits on them.
CHUNK_COLS = [(0, 1536), (1536, 1536), (3072, 1024)]
NCH = len(CHUNK_COLS)           # 3 chunks per n-tile
SLOTS = NT * NCH                # 96 chunk slots
# sa accumulation windows (groups whose contribution sums in PSUM before
# one eviction); later windows are single groups so their sa work lands
# inside the exp stream instead of after it.
WINDOWS = [[0, 1], [2, 3], [4, 5], [6], [7]]


def build_bass():
    nc = bacc.Bacc()

    own_d = nc.dram_tensor("own16", [C, N], F16, kind="ExternalInput")
    oth_d = nc.dram_tensor("oth16", [C, N], F16, kind="ExternalInput")
    # wf/wg are pre-tiled 4x along their free dim so the conv matmul writes
    # all four partition-quad replicas directly (no SBUF-SBUF copy DMAs).
    wf_d = nc.dram_tensor("wf_t", [C, 128], F16, kind="ExternalInput")
    wg_d = nc.dram_tensor("wg_t", [C, 128], F16, kind="ExternalInput")
    wh_d = nc.dram_tensor("wh_t", [C, C], F16, kind="ExternalInput")
    bf_d = nc.dram_tensor("bf_rep", [128, 1], F32, kind="ExternalInput")
    bg_d = nc.dram_tensor("bg_rep", [128, 1], F32, kind="ExternalInput")
    bh_d = nc.dram_tensor("bh_row", [1, C], F16, kind="ExternalInput")
    gm_d = nc.dram_tensor("gamma_rep", [128, 1], F32, kind="ExternalInput")
    on_d = nc.dram_tensor("ones_row", [1, 128], F16, kind="ExternalInput")
    k0_d = nc.dram_tensor("k0_col", [128, 1], F32, kind="ExternalInput")
    out_d = nc.dram_tensor("out", [C, N], F32, kind="ExternalOutput")

    with tile.TileContext(nc) as tc:
        with (
            tc.tile_pool(name="singles", bufs=1) as singles,
            tc.tile_pool(name="own", bufs=1) as ownp,
            tc.tile_pool(name="othp", bufs=1) as othp,
            tc.tile_pool(name="e3", bufs=28) as e3pool,
            tc.tile_pool(name="e2", bufs=14) as e2pool,
            tc.tile_pool(name="fp", bufs=8) as fpool,
            tc.tile_pool(name="zpool", bufs=4) as zpool,
            tc.tile_pool(name="outp", bufs=2) as outp,
            # ps_c (2 banks) serves the convs early and the sa units later;
            # ps_s double-buffers the 3-bank S chunks (6 banks).
            tc.tile_pool(name="ps_c", bufs=2, space="PSUM") as ps_c,
            tc.tile_pool(name="ps_s", bufs=2, space="PSUM") as ps_s,
        ):
            # ---- DMA priority: what the first convs need comes first ----
            wf_sb = [singles.tile([128, 128], F16, name=f"wf{k}") for k in range(2)]
            wg_sb = [singles.tile([128, 128], F16, name=f"wg{k}") for k in range(2)]
            wh_sb = [singles.tile([128, C], F16, name=f"wh{k}") for k in range(2)]
            bf_sb = singles.tile([128, 1], F32)
            bg_sb = singles.tile([128, 1], F32)
            bh_sb = singles.tile([1, C], F16)
            gm_sb = singles.tile([128, 1], F32)
            ones_sb = singles.tile([1, 128], F16)
            k0_sb = singles.tile([128, 1], F32)
            # inputs split [0:1536 | 1536:4096] (first piece covers the g/f
            # convs the first exp chunk needs); the k=0 half issues on the
            # sync queue and k=1 on the scalar queue so the ~620ns-per-DMA
            # issue serialization halves.
            own_sb = [(ownp.tile([128, 1536], F16, name=f"ownA{k}"),
                       ownp.tile([128, N - 1536], F16, name=f"ownB{k}"))
                      for k in range(2)]
            oth_sb = [(othp.tile([128, 1536], F16, name=f"othA{k}"),
                       othp.tile([128, N - 1536], F16, name=f"othB{k}"))
                      for k in range(2)]

            def islice(tr, c0, w):
                if c0 < 1536:
                    return tr[0][:, c0:c0 + w]
                return tr[1][:, c0 - 1536:c0 - 1536 + w]

            dmaq = [nc.sync, nc.scalar]
            for k in range(2):
                dmaq[k].dma_start(out=wg_sb[k], in_=wg_d[128 * k:128 * (k + 1), :])
            for k in range(2):
                dmaq[k].dma_start(out=own_sb[k][0], in_=own_d[128 * k:128 * (k + 1), 0:1536])
            nc.sync.dma_start(out=bg_sb, in_=bg_d[:, :])
            nc.scalar.dma_start(out=bf_sb, in_=bf_d[:, :])
            for k in range(2):
                dmaq[k].dma_start(out=wf_sb[k], in_=wf_d[128 * k:128 * (k + 1), :])
            for k in range(2):
                dmaq[k].dma_start(out=oth_sb[k][0], in_=oth_d[128 * k:128 * (k + 1), 0:1536])
            nc.sync.dma_start(out=k0_sb, in_=k0_d[:, :])
            for k in range(2):
                dmaq[k].dma_start(out=own_sb[k][1], in_=own_d[128 * k:128 * (k + 1), 1536:N])
            for k in range(2):
                dmaq[k].dma_start(out=wh_sb[k], in_=wh_d[128 * k:128 * (k + 1), :])
            nc.sync.dma_start(out=bh_sb, in_=bh_d[:, :])
            nc.scalar.dma_start(out=ones_sb, in_=on_d[:, :])
            nc.sync.dma_start(out=gm_sb, in_=gm_d[:, :])
            for k in range(2):
                dmaq[k].dma_start(out=oth_sb[k][1], in_=oth_d[128 * k:128 * (k + 1), 1536:N])

            # g blocks static (live for the whole kernel); f blocks rotate
            # (f_q[g] is only read during group g's stats chunks).
            g_q = [singles.tile([128, MB], F16, name=f"g{nb}") for nb in range(NMB)]
            f_q = {}
            sa_sb = [singles.tile([128, N], F16, name=f"sa{k}") for k in range(2)]
            hxz = [singles.tile([128, C], BF16, name=f"hxz{i}") for i in range(NT)]
            e_t = {}   # (g, a, h) -> E tile [128, HALF] bf16

            def conv_g(nb, pool=None):
                ps = (pool or ps_c).tile([128, MB], F32, tag="c")
                for k in range(2):
                    nc.tensor.matmul(
                        out=ps,
                        lhsT=wg_sb[k],
                        rhs=islice(own_sb[k], MB * nb, MB),
                        start=(k == 0),
                        stop=(k == 1),
                    )
                nc.vector.tensor_scalar(
                    out=g_q[nb], in0=ps,
                    scalar1=bg_sb[:, 0:1], scalar2=None, op0=ALU.add)

            def conv_f(nb):
                dst = fpool.tile([128, MB], F16, name=f"f{nb}", tag="f")
                f_q[nb] = dst
                ps = ps_c.tile([128, MB], F32, tag="c")
                for k in range(2):
                    nc.tensor.matmul(
                        out=ps,
                        lhsT=wf_sb[k],
                        rhs=islice(oth_sb[k], MB * nb, MB),
                        start=(k == 0),
                        stop=(k == 1),
                    )
                nc.vector.tensor_scalar(
                    out=dst, in0=ps,
                    scalar1=bf_sb[:, 0:1], scalar2=None, op0=ALU.add)

            def conv_h(i):
                ph = ps_c.tile([128, C], F32, tag="c")
                nc.tensor.matmul(out=ph, lhsT=ones_sb, rhs=bh_sb,
                                 start=True, stop=False)
                for k in range(2):
                    nc.tensor.matmul(
                        out=ph,
                        lhsT=islice(own_sb[k], 128 * i, 128),
                        rhs=wh_sb[k],
                        start=False,
                        stop=(k == 1),
                    )
                nc.vector.tensor_copy(out=hxz[i], in_=ph)

            def stats_chunk(i, c, zp):
                """S chunk (n-tile i, m chunk c) -> exp -> E; Z part via
                DVE (c=0,2) or gpsimd (c=1) in-place identity accum."""
                g, a = i // NGROUP, i % NGROUP
                o = 128 * a
                m0, cols = CHUNK_COLS[c]
                sp = ps_s.tile([128, cols], F32, tag="s")
                for mb in range(m0 // MB, (m0 + cols) // MB):
                    j = mb % 4
                    nc.tensor.matmul(
                        out=sp[:, MB * mb - m0:MB * (mb + 1) - m0],
                        lhsT=f_q[g][32 * j:32 * (j + 1), o:o + 128],
                        rhs=g_q[mb][32 * j:32 * (j + 1), :],
                        start=True,
                        stop=True,
                        tile_position=(32 * j, 0),
                    )
                pool = e3pool if cols == 1536 else e2pool
                et = pool.tile([128, cols], BF16, name=f"e{i}_{c}", tag="e")
                e_t[(i, c)] = et
                nc.scalar.activation(out=et, in_=sp, func=AF.Exp,
                                     bias=k0_sb[:, 0:1],
                                     accum_out=zp[:, NCH * a + c:NCH * a + c + 1])

            def zprep(g, zp):
                """Z = sum of the three chunk sums; hxz *= 1/Z (in place)."""
                zt = zpool.tile([128, NGROUP], F32, tag="zt")
                rz = zpool.tile([128, NGROUP], F32, tag="rz")
                nc.vector.tensor_add(out=zt, in0=zp[:, 0:12:3], in1=zp[:, 1:12:3])
                nc.vector.tensor_add(out=zt, in0=zt, in1=zp[:, 2:12:3])
                nc.vector.reciprocal(out=rz, in_=zt)
                for a in range(NGROUP):
                    nc.vector.tensor_scalar(
                        out=hxz[NGROUP * g + a],
                        in0=hxz[NGROUP * g + a],
                        scalar1=rz[:, a:a + 1],
                        scalar2=None,
                        op0=ALU.mult,
                    )

            def sa_unit(w, mb, ch):
                """One window's contribution to sa[:, mb block], half ch."""
                c = mb * MB // 1536          # which E chunk holds this m-block
                m0 = mb * MB - CHUNK_COLS[c][0]
                groups = WINDOWS[w]
                pa = ps_c.tile([128, MB], F32, tag="c")
                nmm = 4 * len(groups)
                k = 0
                for g in groups:
                    for a in range(NGROUP):
                        nc.tensor.matmul(
                            out=pa,
                            lhsT=hxz[NGROUP * g + a][:, 128 * ch:128 * (ch + 1)],
                            rhs=e_t[(NGROUP * g + a, c)][:, m0:m0 + MB],
                            start=(k == 0),
                            stop=(k == nmm - 1),
                        )
                        k += 1
                dst = sa_sb[ch][:, MB * mb:MB * (mb + 1)]
                if w == 0:
                    nc.vector.tensor_scalar(
                        out=dst, in0=pa,
                        scalar1=gm_sb[:, 0:1], scalar2=None, op0=ALU.mult)
                elif w == len(WINDOWS) - 1:
                    # final window: sa_sb already holds gamma*partial + own
                    # (residual pre-added after the previous window), so one
                    # STT finishes the output tile.
                    ot = outp.tile([128, MB], F32, tag="ot")
                    nc.vector.scalar_tensor_tensor(
                        out=ot, in0=pa, scalar=gm_sb[:, 0:1], in1=dst,
                        op0=ALU.mult, op1=ALU.add)
                    nc.sync.dma_start(
                        out=out_d[128 * ch:128 * (ch + 1), MB * mb:MB * (mb + 1)],
                        in_=ot)
                else:
                    nc.vector.scalar_tensor_tensor(
                        out=dst, in0=pa, scalar=gm_sb[:, 0:1], in1=dst,
                        op0=ALU.mult, op1=ALU.add)
                    if w == len(WINDOWS) - 2:
                        nc.vector.tensor_add(
                            out=dst, in0=dst,
                            in1=islice(own_sb[ch], MB * mb, MB))

            # ---- slot schedule ----
            # upfront: chunk (0,0) needs g0-2 + f0; the rest of the first
            # n-tile's chunks need all of g.
            for nb in range(3):
                conv_g(nb)
            conv_f(0)
            for nb in range(3, NMB):
                conv_g(nb)

            # early filler: h and f convs, 2 per slot.  h convs for group
            # g's n-tiles must land before zprep(g) (slot 12g+12); f_q[g]
            # before slot 12g.
            filler = [("h", i) for i in range(4)] \
                   + [("f", nb) for nb in range(1, 4)] \
                   + [("h", i) for i in range(4, 16)] \
                   + [("f", 4), ("f", 5)] \
                   + [("h", i) for i in range(16, 24)] \
                   + [("f", 6), ("f", 7)] \
                   + [("h", i) for i in range(24, NT)]
            FILL_PER_SLOT = 2

            # sa units become available per window after its last zprep.
            # Pace them with a cumulative quota that spreads each window's
            # 16 units across the slots until the NEXT window opens (plus a
            # few slots of overlap) so the PE never drains dry right at a
            # window boundary (which re-throttles the HAM clock).
            ready_slot = {w: 12 * (max(gs) + 1) for w, gs in enumerate(WINDOWS)}
            unit_queue = []
            for w in range(len(WINDOWS)):
                for mb in range(NMB):
                    for ch in range(2):
                        unit_queue.append((ready_slot[w], w, mb, ch))
            unit_queue.sort(key=lambda u: u[0])
            uq_pos = 0
            rs_list = sorted(ready_slot.values()) + [SLOTS]
            spans = {}
            for w, r in ready_slot.items():
                nxt = min([x for x in rs_list if x > r] + [SLOTS])
                spans[w] = max(1, nxt + 4 - r)

            def quota(s):
                q = 0.0
                for w, r in ready_slot.items():
                    if s >= r:
                        q += 16.0 * min(1.0, (s - r) / spans[w])
                return q

            def emit_trailing(s):
                """Emit trailing PE work for slot s."""
                nonlocal uq_pos
                for _ in range(FILL_PER_SLOT):
                    if filler:
                        kind, arg = filler.pop(0)
                        if kind == "g":
                            conv_g(arg)
                        elif kind == "f":
                            conv_f(arg)
                        else:
                            conv_h(arg)
                while uq_pos < min(quota(s), len(unit_queue)):
                    rs, w, mb, ch = unit_queue[uq_pos]
                    if rs > s:
                        break
                    sa_unit(w, mb, ch)
                    uq_pos += 1

            zps = {}
            for i in range(NT):
                g, a = i // NGROUP, i % NGROUP
                if a == 0:
                    zps[g] = zpool.tile([128, NCH * NGROUP], F32, tag="zp",
                                        name=f"zp{g}")
                    if g > 0:
                        zprep(g - 1, zps.pop(g - 1))
                for c in range(NCH):
                    s = NCH * i + c
                    emit_trailing(s)
                    stats_chunk(i, c, zps[g])
            zprep(NG - 1, zps.pop(NG - 1))
            # tail: remaining units (last window)
            while uq_pos < len(unit_queue):
                _, w, mb, ch = unit_queue[uq_pos]
                sa_unit(w, mb, ch)
                uq_pos += 1

    if not nc.is_finalized():
        nc.finalize()
    return nc


_NC_CACHE = None


def _get_nc():
    global _NC_CACHE
    if _NC_CACHE is None:
        _NC_CACHE = build_bass()
    return _NC_CACHE


def make_in_maps(**inputs):
    """Build the 8 per-core input maps (core 2b = x-branch, 2b+1 = y-branch)."""
    f = lambda a: np.ascontiguousarray(np.asarray(a), dtype=np.float32)
    h16 = lambda a: np.ascontiguousarray(np.asarray(a), dtype=np.float16)
    x16 = h16(inputs["x"]).reshape(B, C, N)
    y16 = h16(inputs["y"]).reshape(B, C, N)
    Wfx, bfx = h16(inputs["Wfx"]), f(inputs["bfx"])
    Wgx, bgx = h16(inputs["Wgx"]), f(inputs["bgx"])
    Whx, bhx = h16(inputs["Whx"]), h16(inputs["bhx"])
    Wfy, bfy = h16(inputs["Wfy"]), f(inputs["bfy"])
    Wgy, bgy = h16(inputs["Wgy"]), f(inputs["bgy"])
    Why, bhy = h16(inputs["Why"]), h16(inputs["bhy"])
    gamma = f(inputs["gamma"])

    rep4 = lambda b: np.ascontiguousarray(np.tile(b, 4).reshape(128, 1))
    gam = np.ascontiguousarray(np.broadcast_to(gamma.reshape(1, 1), (128, 1)))

    c16 = lambda a: np.ascontiguousarray(a, dtype=np.float16)
    rep4c = lambda w: c16(np.tile(w.T, (1, 4)))   # [C, C8] -> [C, 128]
    branch = {
        "x": dict(
            wf_t=rep4c(Wfy), wg_t=rep4c(Wgx), wh_t=c16(Whx.T),
            bf_rep=rep4(bfy), bg_rep=rep4(bgx), bh_row=c16(bhx.reshape(1, C)),
        ),
        "y": dict(
            wf_t=rep4c(Wfx), wg_t=rep4c(Wgy), wh_t=c16(Why.T),
            bf_rep=rep4(bfx), bg_rep=rep4(bgy), bh_row=c16(bhy.reshape(1, C)),
        ),
    }

    ones_row = np.ones((1, 128), np.float16)
    k0_col = np.full((128, 1), -K0, np.float32)
    in_maps = []
    for b in range(B):
        in_maps.append(dict(own16=x16[b], oth16=y16[b],
                            gamma_rep=gam, ones_row=ones_row, k0_col=k0_col,
                            **branch["x"]))
        in_maps.append(dict(own16=y16[b], oth16=x16[b],
                            gamma_rep=gam, ones_row=ones_row, k0_col=k0_col,
                            **branch["y"]))
    return in_maps


def kernel(**inputs):
    from concourse.bass_utils import run_bass_kernel_spmd

    nc = _get_nc()
    in_maps = make_in_maps(**inputs)
    res = run_bass_kernel_spmd(nc, in_maps, list(range(8))).results
    out_x = np.stack([res[2 * b]["out"] for b in range(B)]).reshape(B, C, H, W)
    out_y = np.stack([res[2 * b + 1]["out"] for b in range(B)]).reshape(B, C, H, W)
    return (out_x, out_y)
